# revision 28
# baseline (speedup 1.0000x reference)
"""Trainium2 Bass kernel for the ODLUE path-flow model (nn_AESUELOGIT).

Math (per reference):
  V[b,l]   = sum_f X[b,l,1+f]*theta[f] + theta_links[l]        (b = day*hour, 96)
  Vf[b,p]  = sum_l V[b,l]*D[l,p] + psc*log(psf[p])
  pf       = per-OD softmax over each OD's 4 consecutive paths
  f[b,p]   = pf * sqrt_q[od(p)]**2
  out[b,l] = relu(sum_p f[b,p]*D[l,p])

Distribution: shard the path axis P=20000 across 8 cores (2500 paths =
625 ODs per core; OD groups of 4 stay device-local). Each core computes
a partial link flow over its paths; host sums partials + relu.

Per-core dataflow:
  The two big matmuls stream D / D^T from HBM exactly once, in fp8-e4m3
  (exact for D's 0/1 entries) with DoubleRow perf mode (K=256 per MM).
  The non-exact operand (V resp. f) is split hi+lo into two fp8 matmuls
  accumulating in the same fp32 PSUM bank, which preserves ~bf16-level
  accuracy at fp8 speed/bytes.
  matmul1: Vf(b,p_loc) with V^T k-pair tiles as lhsT, packed D as rhs,
           + extra K=1 bf16 row (ones x crow) folding psc*log(psf).
  softmax: exp on ACT (no max-subtraction needed: |Vf| <~ 25 in f32),
           grouped sums of 4 on DVE, fast reciprocal, scale by q.
  matmul2: f^T k-pair tiles (PE transpose + fp8 hi/lo) x packed D^T.

Host prep (layout/sharding only): X transposed+packed to partition-major
k-tiles, D cast to fp8 and packed [128, ktile, n] (plus transposed copy),
per-core slices of D/psf/sqrt_q.
"""

import sys
import types

import ml_dtypes
import numpy as np

# --- NTFF profile hook shim (missing antenv.axon_hooks in this image) ---
try:
    import antenv

    if "antenv.axon_hooks" not in sys.modules:
        _m = types.ModuleType("antenv.axon_hooks")
        _state = {}
        _m.set_axon_ntff_profile_hook = lambda h: _state.__setitem__("h", h)
        _m.get_axon_ntff_profile_hook = lambda: _state.get("h")
        sys.modules["antenv.axon_hooks"] = _m
        antenv.axon_hooks = _m
        try:
            from trn_agent_boot.trn_boot import _ntff_profile_via_ctypes

            _m.set_axon_ntff_profile_hook(
                _ntff_profile_via_ctypes("/opt/axon/libaxon_pjrt.so")
            )
        except Exception:
            pass
except Exception:
    pass

import concourse.bass as bass
import concourse.mybir as mybir
import concourse.tile as tile
from concourse import bacc
from concourse.bass import ds, ts
from concourse.bass_utils import run_bass_kernel_spmd
from concourse.masks import make_identity

BF = mybir.dt.bfloat16
F32 = mybir.dt.float32
FP8 = mybir.dt.float8e4
AF = mybir.ActivationFunctionType
ALU = mybir.AluOpType
AX = mybir.AxisListType
DR = mybir.MatmulPerfMode.DoubleRow

NCORES = 8
B = 96           # n_days * n_hours
L = 2000         # links
CH = 5           # X channels (ch 0 = tt_ff, stripped -> theta row 0 is 0)
P = 20000        # paths
PPG = 4          # paths per OD
PL = P // NCORES          # 2500 local paths
GL = PL // PPG            # 625 local ODs
NLT = (L + 127) // 128    # 16 l-tiles (l padded to LP with zero D rows)
NPT = (PL + 127) // 128   # 20 p-tiles (p padded to PLP with zero D^T rows)
PLP = NPT * 128           # 2560 padded local paths (pad: D cols 0, psf 1, q 0)
GLP = PLP // PPG          # 640 padded local ODs
LP = NLT * 128            # 2048 padded links
NPC = 5                   # matmul1 psum chunks
PC = PLP // NPC           # 512 (exactly one PSUM bank, 16B-aligned offsets)
PCG = PC // PPG           # 128 groups per chunk
NLC = 4                   # matmul2 psum chunks
LC = LP // NLC            # 512

_CACHE = {}


def _build_nc():
    if "nc" in _CACHE:
        return _CACHE["nc"]
    nc = bacc.Bacc()

    xp_ext = nc.declare_dram_parameter("xp", [128, NLT, B, CH], BF, isOutput=False)
    tl_ext = nc.declare_dram_parameter("tlp", [128, NLT], F32, isOutput=False)
    th_ext = nc.declare_dram_parameter("th", [1, CH], F32, isOutput=False)
    d_ext = nc.declare_dram_parameter("dloc", [128, NLT, PLP], FP8, isOutput=False)
    dt_ext = nc.declare_dram_parameter("dtloc", [128, NPT, LP], FP8, isOutput=False)
    psf_ext = nc.declare_dram_parameter("psf", [1, PLP], F32, isOutput=False)
    psc_ext = nc.declare_dram_parameter("psc", [1, 1], F32, isOutput=False)
    sq_ext = nc.declare_dram_parameter("sq", [1, GLP], F32, isOutput=False)
    out_ext = nc.declare_dram_parameter("out", [B, L], F32, isOutput=True)

    with tile.TileContext(nc) as tc:
        with (
            tc.tile_pool(name="const", bufs=1) as const,
            tc.tile_pool(name="work", bufs=1) as work,
        ):
            # ---- big loads on sync/HWDGE, small consts on gpsimd/SWDGE ----
            dsb = work.tile([128, NLT, PLP], FP8)
            # progressive X groups (in l-tiles): the first V tiles are needed
            # first, so load/compute them in small chunks up front
            XGRP = [(0, 2), (2, 2), (4, 4), (8, 8)]
            xq_tiles = [
                work.tile([128, n, B, CH], BF, name=f"xq_{q}", tag=f"xq{q}")
                for q, (o, n) in enumerate(XGRP)
            ]
            def _xq(q):
                o, n = XGRP[q]
                nc.sync.dma_start(out=xq_tiles[q], in_=xp_ext[:, o : o + n])
            _xq(0)
            _xq(1)
            for g in range(NLT // 2):
                nc.sync.dma_start(
                    out=dsb[:, 2 * g : 2 * g + 2], in_=d_ext[:, 2 * g : 2 * g + 2]
                )
                if g == 0:
                    _xq(2)
                if g == 1:
                    _xq(3)
            dtsb = work.tile([128, NPT, LP], FP8)
            for g in range(NPT // 2):
                nc.sync.dma_start(
                    out=dtsb[:, 2 * g : 2 * g + 2], in_=dt_ext[:, 2 * g : 2 * g + 2]
                )

            # ---- small constants (SWDGE) ----
            th_sb = const.tile([128, CH], F32)
            nc.gpsimd.dma_start(out=th_sb, in_=th_ext[:].to_broadcast([128, CH]))
            tl_sb = const.tile([128, NLT], F32)
            nc.gpsimd.dma_start(out=tl_sb, in_=tl_ext[:])
            psf_sb = const.tile([1, PLP], F32)
            nc.gpsimd.dma_start(out=psf_sb, in_=psf_ext[:])
            psc_sb = const.tile([1, 1], F32)
            nc.gpsimd.dma_start(out=psc_sb, in_=psc_ext[:])
            sq_sb = const.tile([128, GLP], F32)
            nc.gpsimd.dma_start(out=sq_sb[:B], in_=sq_ext[:].to_broadcast([B, GLP]))
            ones_sb = const.tile([1, B], BF)
            nc.vector.memset(ones_sb, 1.0)
            # warm the ACT Exp table early so exp chunks don't pay the
            # 1.3us table load inside the matmul1->matmul2 bridge
            dummy = const.tile([1, 8], F32)
            nc.vector.memset(dummy, 0.0)
            nc.scalar.activation(out=dummy, in_=dummy, func=AF.Exp)
            ident = const.tile([128, 128], BF)
            make_identity(nc, ident)

            # ---- V^T tiles (l on partitions), packed 8 l-tiles per DVE op ----
            vt = work.tile([128, NLT, B], BF)
            vtf = work.tile([128, NLT, B], F32)
            vthi = work.tile([128, NLT, B], FP8)
            vtlo = work.tile([128, NLT, B], FP8)
            for g in range(4):
                o, n = XGRP[g]
                tsl = slice(o, o + n)
                xq = xq_tiles[g]
                nc.vector.tensor_scalar_mul(
                    vtf[:, tsl], xq[:, :, :, 1], th_sb[:, 1:2]
                )
                for c in (2, 3, 4):
                    nc.vector.scalar_tensor_tensor(
                        out=vtf[:, tsl], in0=xq[:, :, :, c],
                        scalar=th_sb[:, c : c + 1],
                        in1=vtf[:, tsl], op0=ALU.mult, op1=ALU.add,
                    )
                # + theta_links[l] (varies per (partition, t): broadcast over b)
                tl_sl = tl_sb[:, tsl]
                tl_rep = bass.AP(
                    tensor=tl_sl.tensor,
                    offset=tl_sl.offset,
                    ap=[tl_sl.ap[0], tl_sl.ap[1], [0, B]],
                )
                nc.vector.tensor_tensor(
                    out=vt[:, tsl], in0=vtf[:, tsl], in1=tl_rep, op=ALU.add
                )
                # fp8 hi/lo split: vt = vthi + vtlo to ~0.4% of ulp
                if g == 0:
                    nc.vector.tensor_copy(out=vthi[:, tsl], in_=vt[:, tsl])
                else:
                    nc.scalar.copy(out=vthi[:, tsl], in_=vt[:, tsl])
                nc.vector.tensor_sub(vtlo[:, tsl], vt[:, tsl], vthi[:, tsl])

            # crow = psc * ln(psf)  (bf16 row, folded into matmul1 as K=1)
            lnp = const.tile([1, PLP], F32)
            nc.scalar.activation(out=lnp, in_=psf_sb, func=AF.Ln)
            crow = const.tile([1, PLP], BF)
            nc.vector.tensor_scalar_mul(crow, lnp, psc_sb[:, 0:1])

            # qb = sqrt_q**2 broadcast over batch partitions
            qb = const.tile([128, GLP], F32)
            nc.scalar.activation(out=qb[:B], in_=sq_sb[:B], func=AF.Square)

            with tc.tile_pool(name="ps1", bufs=1, space="PSUM") as ps1p:
                ps1 = [
                    ps1p.tile([128, PC], F32, name=f"ps1_{n}", tag=f"b{n}")
                    for n in range(NPC)
                ]
                NG = NLT // 2
                for g in range(NG):
                    gsl = slice(2 * g, 2 * g + 2)
                    for n in range(NPC):
                        nc.tensor.matmul(
                            ps1[n][:B],
                            lhsT=vthi[:, gsl, :],
                            rhs=dsb[:, gsl, ts(n, PC)],
                            start=(g == 0), stop=False, perf_mode=DR,
                        )
                        nc.tensor.matmul(
                            ps1[n][:B],
                            lhsT=vtlo[:, gsl, :],
                            rhs=dsb[:, gsl, ts(n, PC)],
                            start=False, stop=False, perf_mode=DR,
                        )
                        if g == NG - 1:
                            # bias row: Vf += 1 x crow  (bf16, K=1)
                            nc.tensor.matmul(
                                ps1[n][:B], lhsT=ones_sb[:1, :],
                                rhs=crow[:1, ts(n, PC)],
                                start=False, stop=True, skip_group_check=True,
                            )

                # ---- softmax (grouped by 4 consecutive paths) ----
                e_sb = work.tile([128, PLP], F32)
                f_sb = work.tile([128, PLP], BF)
                s_sb = work.tile([128, GLP], F32)
                r_sb = work.tile([128, GLP], F32)
                t_sb = work.tile([128, GLP], F32)
                for n in range(NPC):
                    nc.scalar.activation(
                        out=e_sb[:B, ts(n, PC)], in_=ps1[n][:B], func=AF.Exp
                    )
                    e3 = e_sb[:B, ts(n, PC)].rearrange("p (g w) -> p g w", w=PPG)
                    nc.vector.reduce_sum(
                        out=s_sb[:B, ds(n * PCG, PCG)], in_=e3, axis=AX.X
                    )
                    nc.vector.reciprocal_approx_fast(
                        out=r_sb[:B, ds(n * PCG, PCG)],
                        in_=s_sb[:B, ds(n * PCG, PCG)],
                    )
                    nc.vector.tensor_mul(
                        t_sb[:B, ds(n * PCG, PCG)],
                        r_sb[:B, ds(n * PCG, PCG)],
                        qb[:B, ds(n * PCG, PCG)],
                    )
                    t_sl = t_sb[:B, ds(n * PCG, PCG)]
                    t_rep = bass.AP(
                        tensor=t_sl.tensor,
                        offset=t_sl.offset,
                        ap=[t_sl.ap[0], t_sl.ap[1], [0, PPG]],
                    )
                    f3 = f_sb[:B, ts(n, PC)].rearrange("p (g w) -> p g w", w=PPG)
                    nc.vector.tensor_tensor(out=f3, in0=e3, in1=t_rep, op=ALU.mult)

            # ---- matmul2: out_partial = f @ D^T, via f^T k-pair tiles ----
            fT8 = work.tile([128, NPT, B], FP8)
            with (
                tc.tile_pool(name="psT", bufs=4, space="PSUM") as psTp,
                tc.tile_pool(name="ps2", bufs=1, space="PSUM") as ps2p,
            ):
                ps2 = [
                    ps2p.tile([128, LC], F32, name=f"ps2_{m}", tag=f"c{m}")
                    for m in range(NLC)
                ]
                for j in range(NPT):
                    pT = psTp.tile([128, B], BF)
                    nc.tensor.transpose(
                        pT, f_sb[:B, ds(128 * j, 128)], ident[:B, :B]
                    )
                    nc.scalar.copy(out=fT8[:, j, :], in_=pT)
                    if j % 2 == 1:
                        gsl = slice(j - 1, j + 1)
                        for m in range(NLC):
                            nc.tensor.matmul(
                                ps2[m][:B],
                                lhsT=fT8[:, gsl, :],
                                rhs=dtsb[:, gsl, ts(m, LC)],
                                start=(j == 1), stop=(j == NPT - 1), perf_mode=DR,
                            )
                out_sb = work.tile([128, LP], F32)
                for m in range(NLC):
                    w = min(LC, L - m * LC)
                    nc.vector.tensor_copy(out=out_sb[:B, ts(m, LC)], in_=ps2[m][:B])
                    nc.sync.dma_start(
                        out=out_ext[:, ds(m * LC, w)], in_=out_sb[:B, ds(m * LC, w)]
                    )

    nc.finalize()
    _CACHE["nc"] = nc
    return nc


def _prep_inputs(X, theta, theta_links, sqrt_q, psf, psc_factor, D):
    bf = ml_dtypes.bfloat16
    fp8 = ml_dtypes.float8_e4m3
    f32 = np.float32

    # X packed: xp[p, t, b, c] = X[b, 128t+p, c], zero-padded l -> 2048
    Xf = np.asarray(X, f32).reshape(B, L, CH).transpose(1, 0, 2)  # [L, B, CH]
    Xpad = np.zeros((NLT * 128, B, CH), f32)
    Xpad[:L] = Xf
    xp = np.ascontiguousarray(
        Xpad.reshape(NLT, 128, B, CH).transpose(1, 0, 2, 3)
    ).astype(bf)  # [128, NLT, B, CH]

    tlp = np.zeros((NLT * 128,), f32)
    tlp[:L] = np.asarray(theta_links, f32)
    tlp = np.ascontiguousarray(tlp.reshape(NLT, 128).T)  # [128, NLT]

    th = np.zeros((1, CH), f32)
    th[0, 1:] = np.asarray(theta, f32)

    psc = np.asarray(psc_factor, f32).reshape(1, 1)
    D8 = np.asarray(D, f32).astype(fp8)  # exact for 0/1 entries

    in_maps = []
    for i in range(NCORES):
        pl = slice(i * PL, (i + 1) * PL)
        gl = slice(i * GL, (i + 1) * GL)
        psf_p = np.ones((1, PLP), f32)
        psf_p[0, :PL] = np.asarray(psf, f32)[pl]
        sq_p = np.zeros((1, GLP), f32)
        sq_p[0, :GL] = np.asarray(sqrt_q, f32)[gl]
        dl = D8[:, pl]                                   # [2000, 2500]
        dpad = np.zeros((LP, PLP), fp8)
        dpad[:L, :PL] = dl
        dloc = np.ascontiguousarray(
            dpad.reshape(NLT, 128, PLP).transpose(1, 0, 2)
        )                                                # [128, NLT, PLP]
        dtpad = np.zeros((PLP, LP), fp8)
        dtpad[:PL, :L] = dl.T
        dtloc = np.ascontiguousarray(
            dtpad.reshape(NPT, 128, LP).transpose(1, 0, 2)
        )                                                # [128, NPT, LP]
        in_maps.append(
            dict(
                xp=xp,
                tlp=tlp,
                th=th,
                dloc=dloc,
                dtloc=dtloc,
                psf=psf_p,
                psc=psc,
                sq=sq_p,
            )
        )
    return in_maps


def run_on_cores(inputs, trace=False, **kw):
    """Compile (cached) + run SPMD on 8 cores; returns BassKernelResults."""
    nc = _build_nc()
    in_maps = _prep_inputs(
        inputs["X"], inputs["theta"], inputs["theta_links"], inputs["sqrt_q"],
        inputs["psf"], inputs["psc_factor"], inputs["D"],
    )
    return run_bass_kernel_spmd(
        nc, in_maps, core_ids=list(range(NCORES)), trace=trace, **kw
    )


def kernel(X, theta, theta_links, sqrt_q, psf, psc_factor, D, path_od=None):
    res = run_on_cores(
        dict(X=X, theta=theta, theta_links=theta_links, sqrt_q=sqrt_q,
             psf=psf, psc_factor=psc_factor, D=D)
    )
    acc = np.zeros((B, L), np.float32)
    for r in res.results:
        acc += np.asarray(r["out"], np.float32)
    return np.maximum(acc, 0.0).reshape(4, 24, L)


# revision 29
# speedup vs baseline: 1.1748x; 1.1748x over previous
"""Trainium2 Bass kernel for the ODLUE path-flow model (nn_AESUELOGIT).

Math (per reference):
  V[b,l]   = sum_f X[b,l,1+f]*theta[f] + theta_links[l]        (b = day*hour, 96)
  Vf[b,p]  = sum_l V[b,l]*D[l,p] + psc*log(psf[p])
  pf       = per-OD softmax over each OD's 4 consecutive paths
  f[b,p]   = pf * sqrt_q[od(p)]**2
  out[b,l] = relu(sum_p f[b,p]*D[l,p])

Distribution: shard the path axis P=20000 across 8 cores (2500 paths =
625 ODs per core; OD groups of 4 stay device-local). Each core computes
a partial link flow over its paths; host sums partials + relu.

Per-core dataflow:
  The two big matmuls stream D / D^T from HBM exactly once, in fp8-e4m3
  (exact for D's 0/1 entries) with DoubleRow perf mode (K=256 per MM).
  The non-exact operand (V resp. f) is split hi+lo into two fp8 matmuls
  accumulating in the same fp32 PSUM bank, which preserves ~bf16-level
  accuracy at fp8 speed/bytes.
  matmul1: Vf(b,p_loc) with V^T k-pair tiles as lhsT, packed D as rhs,
           + extra K=1 bf16 row (ones x crow) folding psc*log(psf).
  softmax: exp on ACT (no max-subtraction needed: |Vf| <~ 25 in f32),
           grouped sums of 4 on DVE, fast reciprocal, scale by q.
  matmul2: f^T k-pair tiles (PE transpose + fp8 hi/lo) x packed D^T.

Host prep (layout/sharding only): X transposed+packed to partition-major
k-tiles, D cast to fp8 and packed [128, ktile, n] (plus transposed copy),
per-core slices of D/psf/sqrt_q.
"""

import sys
import types

import ml_dtypes
import numpy as np

# --- NTFF profile hook shim (missing antenv.axon_hooks in this image) ---
try:
    import antenv

    if "antenv.axon_hooks" not in sys.modules:
        _m = types.ModuleType("antenv.axon_hooks")
        _state = {}
        _m.set_axon_ntff_profile_hook = lambda h: _state.__setitem__("h", h)
        _m.get_axon_ntff_profile_hook = lambda: _state.get("h")
        sys.modules["antenv.axon_hooks"] = _m
        antenv.axon_hooks = _m
        try:
            from trn_agent_boot.trn_boot import _ntff_profile_via_ctypes

            _m.set_axon_ntff_profile_hook(
                _ntff_profile_via_ctypes("/opt/axon/libaxon_pjrt.so")
            )
        except Exception:
            pass
except Exception:
    pass

import concourse.bass as bass
import concourse.mybir as mybir
import concourse.tile as tile
from concourse import bacc
from concourse.bass import ds, ts
from concourse.bass_utils import run_bass_kernel_spmd
from concourse.masks import make_identity

BF = mybir.dt.bfloat16
F32 = mybir.dt.float32
FP8 = mybir.dt.float8e4
AF = mybir.ActivationFunctionType
ALU = mybir.AluOpType
AX = mybir.AxisListType
DR = mybir.MatmulPerfMode.DoubleRow

NCORES = 8
B = 96           # n_days * n_hours
L = 2000         # links
CH = 5           # X channels (ch 0 = tt_ff, stripped -> theta row 0 is 0)
P = 20000        # paths
PPG = 4          # paths per OD
PL = P // NCORES          # 2500 local paths
GL = PL // PPG            # 625 local ODs
NLT = (L + 127) // 128    # 16 l-tiles (l padded to LP with zero D rows)
NPT = (PL + 127) // 128   # 20 p-tiles (p padded to PLP with zero D^T rows)
PLP = NPT * 128           # 2560 padded local paths (pad: D cols 0, psf 1, q 0)
GLP = PLP // PPG          # 640 padded local ODs
LP = NLT * 128            # 2048 padded links
NPC = 5                   # matmul1 psum chunks
PC = PLP // NPC           # 512 (exactly one PSUM bank, 16B-aligned offsets)
PCG = PC // PPG           # 128 groups per chunk
NLC = 4                   # matmul2 psum chunks
LC = LP // NLC            # 512

_CACHE = {}


def _build_nc():
    if "nc" in _CACHE:
        return _CACHE["nc"]
    nc = bacc.Bacc()

    xp_ext = nc.declare_dram_parameter("xp", [128, NLT, B, CH], BF, isOutput=False)
    tl_ext = nc.declare_dram_parameter("tlp", [128, NLT], F32, isOutput=False)
    th_ext = nc.declare_dram_parameter("th", [1, CH], F32, isOutput=False)
    d_ext = nc.declare_dram_parameter("dloc", [128, NLT, PLP], FP8, isOutput=False)
    dt_ext = nc.declare_dram_parameter("dtloc", [128, NPT, LP], FP8, isOutput=False)
    psf_ext = nc.declare_dram_parameter("psf", [1, PLP], F32, isOutput=False)
    psc_ext = nc.declare_dram_parameter("psc", [1, 1], F32, isOutput=False)
    sq_ext = nc.declare_dram_parameter("sq", [1, GLP], F32, isOutput=False)
    out_ext = nc.declare_dram_parameter("out", [B, L], F32, isOutput=True)

    with tile.TileContext(nc) as tc:
        with (
            tc.tile_pool(name="const", bufs=1) as const,
            tc.tile_pool(name="work", bufs=1) as work,
        ):
            # ---- big loads on sync/HWDGE, small consts on gpsimd/SWDGE ----
            dsb = work.tile([128, NLT, PLP], FP8)
            # progressive X groups (in l-tiles): the first V tiles are needed
            # first, so load/compute them in small chunks up front
            XGRP = [(0, 2), (2, 2), (4, 4), (8, 8)]
            xq_tiles = [
                work.tile([128, n, B, CH], BF, name=f"xq_{q}", tag=f"xq{q}")
                for q, (o, n) in enumerate(XGRP)
            ]
            def _xq(q):
                o, n = XGRP[q]
                nc.sync.dma_start(out=xq_tiles[q], in_=xp_ext[:, o : o + n])
            _xq(0)
            _xq(1)
            for g in range(NLT // 2):
                nc.sync.dma_start(
                    out=dsb[:, 2 * g : 2 * g + 2], in_=d_ext[:, 2 * g : 2 * g + 2]
                )
                if g == 0:
                    _xq(2)
                if g == 1:
                    _xq(3)
            dtsb = work.tile([128, NPT, LP], FP8)
            for g in range(NPT // 2):
                nc.sync.dma_start(
                    out=dtsb[:, 2 * g : 2 * g + 2], in_=dt_ext[:, 2 * g : 2 * g + 2]
                )

            # ---- small constants (SWDGE) ----
            th_sb = const.tile([128, CH], F32)
            nc.gpsimd.dma_start(out=th_sb, in_=th_ext[:].to_broadcast([128, CH]))
            tl_sb = const.tile([128, NLT], F32)
            nc.gpsimd.dma_start(out=tl_sb, in_=tl_ext[:])
            psf_sb = const.tile([1, PLP], F32)
            nc.gpsimd.dma_start(out=psf_sb, in_=psf_ext[:])
            psc_sb = const.tile([1, 1], F32)
            nc.gpsimd.dma_start(out=psc_sb, in_=psc_ext[:])
            sq_sb = const.tile([128, GLP], F32)
            nc.gpsimd.dma_start(out=sq_sb[:B], in_=sq_ext[:].to_broadcast([B, GLP]))
            ones_sb = const.tile([1, B], BF)
            nc.vector.memset(ones_sb, 1.0)
            # warm the ACT Exp table early so exp chunks don't pay the
            # 1.3us table load inside the matmul1->matmul2 bridge
            dummy = const.tile([1, 8], F32)
            nc.vector.memset(dummy, 0.0)
            nc.scalar.activation(out=dummy, in_=dummy, func=AF.Exp)
            ident = const.tile([128, 128], BF)
            make_identity(nc, ident)

            # ---- V^T tiles (l on partitions), packed 8 l-tiles per DVE op ----
            vt = work.tile([128, NLT, B], BF)
            vtf = work.tile([128, NLT, B], F32)
            vthi = work.tile([128, NLT, B], FP8)
            vtlo = work.tile([128, NLT, B], FP8)
            for g in range(4):
                o, n = XGRP[g]
                tsl = slice(o, o + n)
                xq = xq_tiles[g]
                nc.vector.tensor_scalar_mul(
                    vtf[:, tsl], xq[:, :, :, 1], th_sb[:, 1:2]
                )
                for c in (2, 3, 4):
                    nc.vector.scalar_tensor_tensor(
                        out=vtf[:, tsl], in0=xq[:, :, :, c],
                        scalar=th_sb[:, c : c + 1],
                        in1=vtf[:, tsl], op0=ALU.mult, op1=ALU.add,
                    )
                # + theta_links[l] (varies per (partition, t): broadcast over b)
                tl_sl = tl_sb[:, tsl]
                tl_rep = bass.AP(
                    tensor=tl_sl.tensor,
                    offset=tl_sl.offset,
                    ap=[tl_sl.ap[0], tl_sl.ap[1], [0, B]],
                )
                nc.vector.tensor_tensor(
                    out=vt[:, tsl], in0=vtf[:, tsl], in1=tl_rep, op=ALU.add
                )
                # fp8 hi/lo split: vt = vthi + vtlo to ~0.4% of ulp
                nc.scalar.copy(out=vthi[:, tsl], in_=vt[:, tsl])
                nc.vector.tensor_sub(vtlo[:, tsl], vt[:, tsl], vthi[:, tsl])

            # crow = psc * ln(psf)  (bf16 row, folded into matmul1 as K=1)
            lnp = const.tile([1, PLP], F32)
            nc.scalar.activation(out=lnp, in_=psf_sb, func=AF.Ln)
            crow = const.tile([1, PLP], BF)
            nc.vector.tensor_scalar_mul(crow, lnp, psc_sb[:, 0:1])

            # qb = sqrt_q**2 broadcast over batch partitions
            qb = const.tile([128, GLP], F32)
            nc.scalar.activation(out=qb[:B], in_=sq_sb[:B], func=AF.Square)

            with tc.tile_pool(name="ps1", bufs=1, space="PSUM") as ps1p:
                ps1 = [
                    ps1p.tile([128, PC], F32, name=f"ps1_{n}", tag=f"b{n}")
                    for n in range(NPC)
                ]
                NG = NLT // 2
                for g in range(NG):
                    gsl = slice(2 * g, 2 * g + 2)
                    for n in range(NPC):
                        nc.tensor.matmul(
                            ps1[n][:B],
                            lhsT=vthi[:, gsl, :],
                            rhs=dsb[:, gsl, ts(n, PC)],
                            start=(g == 0), stop=False, perf_mode=DR,
                        )
                        nc.tensor.matmul(
                            ps1[n][:B],
                            lhsT=vtlo[:, gsl, :],
                            rhs=dsb[:, gsl, ts(n, PC)],
                            start=False, stop=False, perf_mode=DR,
                        )
                        if g == NG - 1:
                            # bias row: Vf += 1 x crow  (bf16, K=1)
                            nc.tensor.matmul(
                                ps1[n][:B], lhsT=ones_sb[:1, :],
                                rhs=crow[:1, ts(n, PC)],
                                start=False, stop=True, skip_group_check=True,
                            )

                # ---- softmax (grouped by 4 consecutive paths) ----
                e_sb = work.tile([128, PLP], F32)
                f_sb = work.tile([128, PLP], BF)
                s_sb = work.tile([128, GLP], F32)
                r_sb = work.tile([128, GLP], F32)
                t_sb = work.tile([128, GLP], F32)
                for n in range(NPC):
                    nc.scalar.activation(
                        out=e_sb[:B, ts(n, PC)], in_=ps1[n][:B], func=AF.Exp
                    )
                    e3 = e_sb[:B, ts(n, PC)].rearrange("p (g w) -> p g w", w=PPG)
                    nc.vector.reduce_sum(
                        out=s_sb[:B, ds(n * PCG, PCG)], in_=e3, axis=AX.X
                    )
                    nc.vector.reciprocal_approx_fast(
                        out=r_sb[:B, ds(n * PCG, PCG)],
                        in_=s_sb[:B, ds(n * PCG, PCG)],
                    )
                    nc.vector.tensor_mul(
                        t_sb[:B, ds(n * PCG, PCG)],
                        r_sb[:B, ds(n * PCG, PCG)],
                        qb[:B, ds(n * PCG, PCG)],
                    )
                    t_sl = t_sb[:B, ds(n * PCG, PCG)]
                    t_rep = bass.AP(
                        tensor=t_sl.tensor,
                        offset=t_sl.offset,
                        ap=[t_sl.ap[0], t_sl.ap[1], [0, PPG]],
                    )
                    f3 = f_sb[:B, ts(n, PC)].rearrange("p (g w) -> p g w", w=PPG)
                    nc.vector.tensor_tensor(out=f3, in0=e3, in1=t_rep, op=ALU.mult)

            # ---- matmul2: out_partial = f @ D^T, via f^T k-pair tiles ----
            fT8 = work.tile([128, NPT, B], FP8)
            with (
                tc.tile_pool(name="psT", bufs=3, space="PSUM") as psTp,
                tc.tile_pool(name="ps2", bufs=1, space="PSUM") as ps2p,
            ):
                ps2 = [
                    ps2p.tile([128, LC], F32, name=f"ps2_{m}", tag=f"c{m}")
                    for m in range(NLC)
                ]
                for j in range(NPT):
                    pT = psTp.tile([128, B], BF)
                    nc.tensor.transpose(
                        pT, f_sb[:B, ds(128 * j, 128)], ident[:B, :B]
                    )
                    nc.scalar.copy(out=fT8[:, j, :], in_=pT)
                    if j % 2 == 1:
                        gsl = slice(j - 1, j + 1)
                        for m in range(NLC):
                            nc.tensor.matmul(
                                ps2[m][:B],
                                lhsT=fT8[:, gsl, :],
                                rhs=dtsb[:, gsl, ts(m, LC)],
                                start=(j == 1), stop=(j == NPT - 1), perf_mode=DR,
                            )
                out_sb = work.tile([128, LP], F32)
                for m in range(NLC):
                    w = min(LC, L - m * LC)
                    nc.vector.tensor_copy(out=out_sb[:B, ts(m, LC)], in_=ps2[m][:B])
                    nc.sync.dma_start(
                        out=out_ext[:, ds(m * LC, w)], in_=out_sb[:B, ds(m * LC, w)]
                    )

    nc.finalize()
    _CACHE["nc"] = nc
    return nc


def _prep_inputs(X, theta, theta_links, sqrt_q, psf, psc_factor, D):
    bf = ml_dtypes.bfloat16
    fp8 = ml_dtypes.float8_e4m3
    f32 = np.float32

    # X packed: xp[p, t, b, c] = X[b, 128t+p, c], zero-padded l -> 2048
    Xf = np.asarray(X, f32).reshape(B, L, CH).transpose(1, 0, 2)  # [L, B, CH]
    Xpad = np.zeros((NLT * 128, B, CH), f32)
    Xpad[:L] = Xf
    xp = np.ascontiguousarray(
        Xpad.reshape(NLT, 128, B, CH).transpose(1, 0, 2, 3)
    ).astype(bf)  # [128, NLT, B, CH]

    tlp = np.zeros((NLT * 128,), f32)
    tlp[:L] = np.asarray(theta_links, f32)
    tlp = np.ascontiguousarray(tlp.reshape(NLT, 128).T)  # [128, NLT]

    th = np.zeros((1, CH), f32)
    th[0, 1:] = np.asarray(theta, f32)

    psc = np.asarray(psc_factor, f32).reshape(1, 1)
    D8 = np.asarray(D, f32).astype(fp8)  # exact for 0/1 entries

    in_maps = []
    for i in range(NCORES):
        pl = slice(i * PL, (i + 1) * PL)
        gl = slice(i * GL, (i + 1) * GL)
        psf_p = np.ones((1, PLP), f32)
        psf_p[0, :PL] = np.asarray(psf, f32)[pl]
        sq_p = np.zeros((1, GLP), f32)
        sq_p[0, :GL] = np.asarray(sqrt_q, f32)[gl]
        dl = D8[:, pl]                                   # [2000, 2500]
        dpad = np.zeros((LP, PLP), fp8)
        dpad[:L, :PL] = dl
        dloc = np.ascontiguousarray(
            dpad.reshape(NLT, 128, PLP).transpose(1, 0, 2)
        )                                                # [128, NLT, PLP]
        dtpad = np.zeros((PLP, LP), fp8)
        dtpad[:PL, :L] = dl.T
        dtloc = np.ascontiguousarray(
            dtpad.reshape(NPT, 128, LP).transpose(1, 0, 2)
        )                                                # [128, NPT, LP]
        in_maps.append(
            dict(
                xp=xp,
                tlp=tlp,
                th=th,
                dloc=dloc,
                dtloc=dtloc,
                psf=psf_p,
                psc=psc,
                sq=sq_p,
            )
        )
    return in_maps


def run_on_cores(inputs, trace=False, **kw):
    """Compile (cached) + run SPMD on 8 cores; returns BassKernelResults."""
    nc = _build_nc()
    in_maps = _prep_inputs(
        inputs["X"], inputs["theta"], inputs["theta_links"], inputs["sqrt_q"],
        inputs["psf"], inputs["psc_factor"], inputs["D"],
    )
    return run_bass_kernel_spmd(
        nc, in_maps, core_ids=list(range(NCORES)), trace=trace, **kw
    )


def kernel(X, theta, theta_links, sqrt_q, psf, psc_factor, D, path_od=None):
    res = run_on_cores(
        dict(X=X, theta=theta, theta_links=theta_links, sqrt_q=sqrt_q,
             psf=psf, psc_factor=psc_factor, D=D)
    )
    acc = np.zeros((B, L), np.float32)
    for r in res.results:
        acc += np.asarray(r["out"], np.float32)
    return np.maximum(acc, 0.0).reshape(4, 24, L)
